# revision 68
# baseline (speedup 1.0000x reference)
"""CausalMaskedLinear Trainium2 kernel (v3: head/tail-optimized).

y = x @ (W * mask).T + b with a block-banded causal mask: output block o
(128 rows) attends to input blocks j in [o-7, o], so only 228 of the
1024 128x128 weight blocks are live.

Strategy: data-parallel over batch (8192/8 = 1024 rows per core),
weights/bias replicated.  Per output block o the two OLDEST band blocks
(j = lo, lo+1, for o >= 2) are computed in fp8 e4m3 via one DoubleRow
matmul (two 128-deep contractions per instruction, 2x PE rate); the
remaining blocks run in fp16.  Numerics (validated against the harness
seed): max/scale err 1.39e-2 < 2e-2 gate.

Scaling: e4m3's normal range starts at 2^-6, so x is quantized as
e4m3(8*x) and w as e4m3(256*w); fp16 blocks carry w*2048 so every
matmul contributes 2048*x*w to the shared PSUM accumulation.  Output
stage is split per 512-col half: h0 on the ACT engine
(Identity(psum/2048 + bias)), h1 on DVE (tensor_scalar mult+add), so
neither engine gates PSUM recycling.

v4 changes (trace-driven; v2 baseline 120.7us, v3 regressed to 126.6us):
- v2's head had a 12us PE-idle gap: x block 0 rode the latest-starting
  DMA queue behind bias while bulk x/w saturated the engines, and the
  idle re-throttled HAM to 1.2GHz.  v3 fixed the ordering but used many
  small transfers; each queue has only ~6 in-flight transfer
  semaphores, and an issue that re-arms one WAITS for the prior
  transfer, so the issue pipeline serialized and starved the mid-game.
- v4: few BIG transfers per queue in need-order.  sync queue (earliest
  to start) carries o0/o1 weights then all of x in 2-block chunks;
  scalar carries bias + o2..o7 weights; gpsimd carries all late bulk
  behind a data gate on x block 2.
- Warmup: 6x N=512 + 4x N=128 dummy matmuls end right when x block 0
  lands (~10.4us), keeping HAM warm without delaying real work.
- Output stage split per 512-col half across engines (h0 ACT, h1 DVE)
  and across queues (h0 scalar, h1 gpsimd) so the final o's output
  drains in ~2us instead of 6us queued behind 8MB of writes.
"""

import numpy as np
import ml_dtypes

NUM_TIME_STEPS = 32
IN_FEAT = 128
OUT_FEAT = 128
TRI_BLOCK = 8
BATCH = 8192
N_CORES = 8
BC = BATCH // N_CORES  # batch rows per core

IN_SIZE = NUM_TIME_STEPS * IN_FEAT
OUT_SIZE = NUM_TIME_STEPS * OUT_FEAT

SX = 8.0     # fp8 x scale
SW = 256.0   # fp8 w scale
SCALE = SX * SW  # 2048; fp16 w blocks carry w*SCALE

E4 = ml_dtypes.float8_e4m3  # matches mybir.dt.float8e4


def _band(o):
    return range(max(0, o - TRI_BLOCK + 1), o + 1)


# per-o split: o>=2 -> fp8 pair (lo, lo+1) + fp16 rest; o<2 -> all fp16
def _f16_blocks(o):
    bl = list(_band(o))
    return bl[2:] if o >= 2 else bl


N_F16 = sum(len(_f16_blocks(o)) for o in range(NUM_TIME_STEPS))  # 168
_K16 = np.cumsum([0] + [len(_f16_blocks(o)) for o in range(NUM_TIME_STEPS)])
N_PAIR = NUM_TIME_STEPS - 2  # 30

_PROGRAM = None


def _build_program():
    import concourse.bacc as bacc
    import concourse.bass as bass
    import concourse.mybir as mybir
    import concourse.tile as tile

    f32 = mybir.dt.float32
    f16 = mybir.dt.float16
    f8 = mybir.dt.float8e4

    nc = bacc.Bacc("TRN2", target_bir_lowering=False, debug=False,
                   enable_asserts=False)

    x16_d = nc.dram_tensor("x16", [128, NUM_TIME_STEPS * BC], f16,
                           kind="ExternalInput")
    w16_d = nc.dram_tensor("w16", [128, N_F16, 128], f16,
                           kind="ExternalInput")
    w8_d = nc.dram_tensor("w8", [128, N_PAIR, 2, 128], f8,
                          kind="ExternalInput")
    bias_d = nc.dram_tensor("bias_t", [128, NUM_TIME_STEPS], f32,
                            kind="ExternalInput")
    yT_d = nc.dram_tensor("yT", [NUM_TIME_STEPS, 128, BC], f16,
                          kind="ExternalOutput")

    with tile.TileContext(nc) as tc:
        with (
            tc.tile_pool(name="xp16", bufs=1) as xp16,
            tc.tile_pool(name="xp8", bufs=1) as xp8,
            tc.tile_pool(name="wp16", bufs=1) as wp16,
            tc.tile_pool(name="wp8", bufs=1) as wp8,
            tc.tile_pool(name="op", bufs=16) as op,
            tc.tile_pool(name="wmp", bufs=1) as wmp,
            tc.tile_pool(name="bp", bufs=1) as bp,
            tc.tile_pool(name="psp", bufs=8, space=bass.MemorySpace.PSUM) as psp,
        ):
            bias_t = bp.tile([128, NUM_TIME_STEPS], f32)

            # PE pre-warm: HAM un-throttles (1.2 -> 2.4 GHz) only after
            # ~3.4us sustained activity; sized to end right as x block 0
            # lands (~10.4us) so the PE never idles long enough to
            # re-throttle.  Also preload the ACT table set (~2.7us
            # one-time) before the first output op needs it.
            warm_in = wmp.tile([128, 512], f16, tag="warm")
            nc.gpsimd.memset(warm_in[:], 0.0)
            warm_out = wmp.tile([128, 1], f32, tag="warmo")
            nc.scalar.activation(warm_out[:], warm_in[:, :1],
                                 mybir.ActivationFunctionType.Identity,
                                 bias=0.0, scale=1.0)
            warm_ps = psp.tile([128, 512], f32, tag="ps")
            for _ in range(8):
                nc.tensor.matmul(warm_ps[:], warm_in[:, :128], warm_in[:],
                                 start=True, stop=True)
            for _ in range(4):
                nc.tensor.matmul(warm_ps[:, :128], warm_in[:, :128],
                                 warm_in[:, :128], start=True, stop=True)

            # big region-tracked tiles; per-block DMAs keep deps fine-grained
            x16_t = xp16.tile([128, NUM_TIME_STEPS * BC], f16, tag="x16")
            w16_t = wp16.tile([128, N_F16, 128], f16, tag="w16")
            w8_t = wp8.tile([128, N_PAIR, 2, 128], f8, tag="w8")

            def ld_x(eng, j0, j1):
                eng.dma_start(x16_t[:, j0 * BC:j1 * BC],
                              x16_d[:, j0 * BC:j1 * BC])

            def ld_w16(eng, k0, k1):
                eng.dma_start(w16_t[:, k0:k1, :], w16_d[:, k0:k1, :])

            def ld_w8(eng, p0, p1):
                eng.dma_start(w8_t[:, p0:p1, :, :], w8_d[:, p0:p1, :, :])

            # DMA design rules (v3 post-mortem): each queue has only ~6
            # in-flight transfer semaphores -- an issue instruction that
            # re-arms a semaphore WAITS for its previous transfer to
            # complete, so the issue pipeline is completion-gated.  Use
            # FEW, BIG transfers (>=2KB per-partition lines) per queue,
            # in need-order, and keep in-loop writes off the queues that
            # must keep delivering bulk.

            # SYNC queue (first to start moving, ~8.6us): o=0/o=1 weights
            # then all of x in 2-block chunks, in need-order.  w16 block
            # ranges per o: o0=[0:1], o1=[1:3], o2=[3:4], o3=[4:6],
            # o4=[6:9], o5=[9:13], o6=[13:18], o7=[18:24], o8+=[24+6(o-8)].
            ld_w16(nc.sync, 0, 3)
            nc.sync.dma_start(x16_t[:, 0:512], x16_d[:, 0:512])
            nc.sync.dma_start(x16_t[:, 512:1024], x16_d[:, 512:1024])
            ld_x(nc.sync, 1, 3)
            for j in range(3, NUM_TIME_STEPS, 2):  # pairs (3,4),(5,6)..
                ld_x(nc.sync, j, min(j + 2, NUM_TIME_STEPS))

            # SCALAR queue (~10.4us): bias + early-need small weights.
            nc.scalar.dma_start(bias_t[:], bias_d[:])
            ld_w8(nc.scalar, 0, 2)
            ld_w16(nc.scalar, 3, 9)
            ld_w8(nc.scalar, 2, 8)
            ld_w16(nc.scalar, 9, 24)

            # GPSIMD queue: all late bulk, gated on x block 1 so it stays
            # out of the contended early window (earliest need is o=8 at
            # ~22us).  The tile scheduler is dependency-driven, so a
            # standalone gate op gets hoisted past the dma_starts; instead
            # poke one element of each gated destination region (reading
            # the x16 gate column) to create a real WAW dependency the
            # scheduler must honor.  The DMA then overwrites the poked
            # element with the true data.
            gx = x16_t[:, BC:BC + 1]
            nc.gpsimd.tensor_scalar_add(w16_t[:, 24, 0:1], gx, 0.0)
            ld_w16(nc.gpsimd, 24, 44)
            nc.gpsimd.tensor_scalar_add(w8_t[:, 8, 0, 0:1], gx, 0.0)
            ld_w8(nc.gpsimd, 8, 16)
            nc.gpsimd.tensor_scalar_add(w16_t[:, 44, 0:1], gx, 0.0)
            ld_w16(nc.gpsimd, 44, 64)
            nc.gpsimd.tensor_scalar_add(w16_t[:, 64, 0:1], gx, 0.0)
            ld_w16(nc.gpsimd, 64, 112)
            nc.gpsimd.tensor_scalar_add(w8_t[:, 16, 0, 0:1], gx, 0.0)
            ld_w8(nc.gpsimd, 16, 30)
            nc.gpsimd.tensor_scalar_add(w16_t[:, 112, 0:1], gx, 0.0)
            ld_w16(nc.gpsimd, 112, 168)

            # x8 derived on-device: e4m3(8 * x16) per block on DVE
            # (gpsimd measures ~13us per block for this op -- unusable;
            # per-pair tiles split DVE/ACT measured slower too).
            x8_t = xp8.tile([128, NUM_TIME_STEPS, BC], f8, tag="x8")

            def convert(j):
                nc.vector.tensor_scalar_mul(
                    x8_t[:, j, :], x16_t[:, j * BC:(j + 1) * BC], 8.0)

            convert(0)
            convert(1)

            inv = 1.0 / SCALE
            max_x8 = NUM_TIME_STEPS - TRI_BLOCK + 1  # highest x8 block read
            for o in range(NUM_TIME_STEPS):
                # x8[j] is first read at o = j+6 (as lo+1).  The x8 tile
                # is watermark-tracked: each DR matmul waits on the
                # latest x8 write issued before it in program order, so
                # issue conv(j) at the top of iteration j+5 -- a full
                # iteration before its first reader, and none at all
                # during the o<7 ramp.
                if 2 <= o - 5 <= max_x8:
                    convert(o - 5)
                lo = max(0, o - TRI_BLOCK + 1)
                f16bl = _f16_blocks(o)
                k0 = int(_K16[o])
                ps = [psp.tile([128, 512], f32, tag="ps", name=f"ps{o}_{h}")
                      for h in range(2)]
                started = [False, False]
                n_units = (1 if o >= 2 else 0) + len(f16bl)
                unit = 0
                if o >= 2:
                    unit += 1
                    for h in range(2):
                        nc.tensor.matmul(
                            ps[h][:],
                            w8_t[:, o - 2, :, :],
                            x8_t[:, lo:lo + 2, h * 512:(h + 1) * 512],
                            start=True, stop=(unit == n_units),
                            perf_mode=mybir.MatmulPerfMode.DoubleRow)
                    started = [True, True]
                for idx, j in enumerate(f16bl):
                    unit += 1
                    for h in range(2):
                        nc.tensor.matmul(
                            ps[h][:],
                            w16_t[:, k0 + idx, :],
                            x16_t[:, j * BC + h * 512:j * BC + (h + 1) * 512],
                            start=not started[h], stop=(unit == n_units))
                    started = [True, True]
                # output stage split across engines: h0 on ACT, h1 on DVE
                out0 = op.tile([128, 512], f16, tag="oh0")
                out1 = op.tile([128, 512], f16, tag="oh1")
                nc.scalar.activation(out0[:], ps[0][:],
                                     mybir.ActivationFunctionType.Identity,
                                     bias=bias_t[:, o:o + 1], scale=inv)
                nc.vector.tensor_scalar(
                    out1[:], ps[1][:], inv, bias_t[:, o:o + 1],
                    mybir.AluOpType.mult, mybir.AluOpType.add)
                nc.scalar.dma_start(yT_d[o, :, 0:512], out0[:])
                nc.sync.dma_start(yT_d[o, :, 512:1024], out1[:])

    nc.compile()
    return nc


def _get_program():
    global _PROGRAM
    if _PROGRAM is None:
        _PROGRAM = _build_program()
    return _PROGRAM


def _pack_inputs(x, weight, bias, mask):
    x = np.asarray(x, dtype=np.float32)
    weight = np.asarray(weight, dtype=np.float32)
    bias = np.asarray(bias, dtype=np.float32)
    mask = np.asarray(mask)
    wm = weight * mask

    w16_flat = np.empty((128, N_F16 * 128), dtype=np.float16)
    k = 0
    for o in range(NUM_TIME_STEPS):
        for j in _f16_blocks(o):
            blk = wm[o * 128:(o + 1) * 128, j * 128:(j + 1) * 128]
            w16_flat[:, k * 128:(k + 1) * 128] = (blk.T * SCALE).astype(
                np.float16)
            k += 1

    w8_flat = np.empty((128, N_PAIR * 256), dtype=E4)
    for o in range(2, NUM_TIME_STEPS):
        lo = max(0, o - TRI_BLOCK + 1)
        p = o - 2
        for s, j in enumerate((lo, lo + 1)):
            blk = wm[o * 128:(o + 1) * 128, j * 128:(j + 1) * 128]
            w8_flat[:, p * 256 + s * 128:p * 256 + (s + 1) * 128] = (
                blk.T * SW).astype(E4)

    bias_t = np.ascontiguousarray(bias.reshape(NUM_TIME_STEPS, 128).T)

    x16 = x.astype(np.float16)
    in_maps = []
    for c in range(N_CORES):
        sl = slice(c * BC, (c + 1) * BC)
        x16c = np.ascontiguousarray(
            x16[sl].reshape(BC, NUM_TIME_STEPS, 128).transpose(2, 1, 0)
        ).reshape(128, NUM_TIME_STEPS * BC)
        in_maps.append({
            "x16": x16c,
            "w16": w16_flat.reshape(128, N_F16, 128),
            "w8": w8_flat.reshape(128, N_PAIR, 2, 128),
            "bias_t": bias_t,
        })
    return in_maps


def _run(inputs, trace=False):
    from concourse.bass_utils import run_bass_kernel_spmd

    nc = _get_program()
    in_maps = _pack_inputs(**inputs)
    res = run_bass_kernel_spmd(nc, in_maps, list(range(N_CORES)), trace=trace)

    y = np.empty((BATCH, OUT_SIZE), dtype=np.float32)
    for c in range(N_CORES):
        yTc = res.results[c]["yT"].astype(np.float32).reshape(OUT_SIZE, BC)
        y[c * BC:(c + 1) * BC] = yTc.T
    return y, res


def kernel(x, weight, bias, mask):
    y, _ = _run({"x": x, "weight": weight, "bias": bias, "mask": mask})
    return y


# revision 70
# speedup vs baseline: 1.1159x; 1.1159x over previous
"""CausalMaskedLinear Trainium2 kernel (v3: head/tail-optimized).

y = x @ (W * mask).T + b with a block-banded causal mask: output block o
(128 rows) attends to input blocks j in [o-7, o], so only 228 of the
1024 128x128 weight blocks are live.

Strategy: data-parallel over batch (8192/8 = 1024 rows per core),
weights/bias replicated.  Per output block o the two OLDEST band blocks
(j = lo, lo+1, for o >= 2) are computed in fp8 e4m3 via one DoubleRow
matmul (two 128-deep contractions per instruction, 2x PE rate); the
remaining blocks run in fp16.  Numerics (validated against the harness
seed): max/scale err 1.39e-2 < 2e-2 gate.

Scaling: e4m3's normal range starts at 2^-6, so x is quantized as
e4m3(8*x) and w as e4m3(256*w); fp16 blocks carry w*2048 so every
matmul contributes 2048*x*w to the shared PSUM accumulation.  Output
stage is split per 512-col half: h0 on the ACT engine
(Identity(psum/2048 + bias)), h1 on DVE (tensor_scalar mult+add), so
neither engine gates PSUM recycling.

v4 changes (trace-driven; v2 baseline 120.7us, v3 regressed to 126.6us):
- v2's head had a 12us PE-idle gap: x block 0 rode the latest-starting
  DMA queue behind bias while bulk x/w saturated the engines, and the
  idle re-throttled HAM to 1.2GHz.  v3 fixed the ordering but used many
  small transfers; each queue has only ~6 in-flight transfer
  semaphores, and an issue that re-arms one WAITS for the prior
  transfer, so the issue pipeline serialized and starved the mid-game.
- v4: few BIG transfers per queue in need-order.  sync queue (earliest
  to start) carries o0/o1 weights then all of x in 2-block chunks;
  scalar carries bias + o2..o7 weights; gpsimd carries all late bulk
  behind a data gate on x block 2.
- Warmup: 6x N=512 + 4x N=128 dummy matmuls end right when x block 0
  lands (~10.4us), keeping HAM warm without delaying real work.
- Output stage split per 512-col half across engines (h0 ACT, h1 DVE)
  and across queues (h0 scalar, h1 gpsimd) so the final o's output
  drains in ~2us instead of 6us queued behind 8MB of writes.
"""

import numpy as np
import ml_dtypes

NUM_TIME_STEPS = 32
IN_FEAT = 128
OUT_FEAT = 128
TRI_BLOCK = 8
BATCH = 8192
N_CORES = 8
BC = BATCH // N_CORES  # batch rows per core

IN_SIZE = NUM_TIME_STEPS * IN_FEAT
OUT_SIZE = NUM_TIME_STEPS * OUT_FEAT

SX = 8.0     # fp8 x scale
SW = 256.0   # fp8 w scale
SCALE = SX * SW  # 2048; fp16 w blocks carry w*SCALE

E4 = ml_dtypes.float8_e4m3  # matches mybir.dt.float8e4


def _band(o):
    return range(max(0, o - TRI_BLOCK + 1), o + 1)


# per-o split: o>=2 -> fp8 pair (lo, lo+1) + fp16 rest; o<2 -> all fp16
def _f16_blocks(o):
    bl = list(_band(o))
    return bl[2:] if o >= 2 else bl


N_F16 = sum(len(_f16_blocks(o)) for o in range(NUM_TIME_STEPS))  # 168
_K16 = np.cumsum([0] + [len(_f16_blocks(o)) for o in range(NUM_TIME_STEPS)])
N_PAIR = NUM_TIME_STEPS - 2  # 30

_PROGRAM = None


def _build_program():
    import concourse.bacc as bacc
    import concourse.bass as bass
    import concourse.mybir as mybir
    import concourse.tile as tile

    f32 = mybir.dt.float32
    f16 = mybir.dt.float16
    f8 = mybir.dt.float8e4

    nc = bacc.Bacc("TRN2", target_bir_lowering=False, debug=False,
                   enable_asserts=False)

    x16_d = nc.dram_tensor("x16", [128, NUM_TIME_STEPS * BC], f16,
                           kind="ExternalInput")
    w16_d = nc.dram_tensor("w16", [128, N_F16, 128], f16,
                           kind="ExternalInput")
    w8_d = nc.dram_tensor("w8", [128, N_PAIR, 2, 128], f8,
                          kind="ExternalInput")
    bias_d = nc.dram_tensor("bias_t", [128, NUM_TIME_STEPS], f32,
                            kind="ExternalInput")
    yT_d = nc.dram_tensor("yT", [NUM_TIME_STEPS, 128, BC], f16,
                          kind="ExternalOutput")

    with tile.TileContext(nc) as tc:
        with (
            tc.tile_pool(name="xp16", bufs=1) as xp16,
            tc.tile_pool(name="xp8", bufs=1) as xp8,
            tc.tile_pool(name="wp16", bufs=1) as wp16,
            tc.tile_pool(name="wp8", bufs=1) as wp8,
            tc.tile_pool(name="op", bufs=16) as op,
            tc.tile_pool(name="wmp", bufs=1) as wmp,
            tc.tile_pool(name="bp", bufs=1) as bp,
            tc.tile_pool(name="psp", bufs=8, space=bass.MemorySpace.PSUM) as psp,
        ):
            bias_t = bp.tile([128, NUM_TIME_STEPS], f32)

            # PE pre-warm: HAM un-throttles (1.2 -> 2.4 GHz) only after
            # ~3.4us sustained activity; sized to end right as x block 0
            # lands (~10.4us) so the PE never idles long enough to
            # re-throttle.  Also preload the ACT table set (~2.7us
            # one-time) before the first output op needs it.
            warm_in = wmp.tile([128, 512], f16, tag="warm")
            nc.gpsimd.memset(warm_in[:], 0.0)
            warm_out = wmp.tile([128, 1], f32, tag="warmo")
            nc.scalar.activation(warm_out[:], warm_in[:, :1],
                                 mybir.ActivationFunctionType.Identity,
                                 bias=0.0, scale=1.0)
            warm_ps = psp.tile([128, 512], f32, tag="ps")
            for _ in range(8):
                nc.tensor.matmul(warm_ps[:], warm_in[:, :128], warm_in[:],
                                 start=True, stop=True)
            for _ in range(4):
                nc.tensor.matmul(warm_ps[:, :128], warm_in[:, :128],
                                 warm_in[:, :128], start=True, stop=True)

            # big region-tracked tiles; per-block DMAs keep deps fine-grained
            x16_t = xp16.tile([128, NUM_TIME_STEPS * BC], f16, tag="x16")
            w16_t = wp16.tile([128, N_F16, 128], f16, tag="w16")
            w8_t = wp8.tile([128, N_PAIR, 2, 128], f8, tag="w8")

            def ld_x(eng, j0, j1):
                eng.dma_start(x16_t[:, j0 * BC:j1 * BC],
                              x16_d[:, j0 * BC:j1 * BC])

            def ld_w16(eng, k0, k1):
                eng.dma_start(w16_t[:, k0:k1, :], w16_d[:, k0:k1, :])

            def ld_w8(eng, p0, p1):
                eng.dma_start(w8_t[:, p0:p1, :, :], w8_d[:, p0:p1, :, :])

            # DMA design rules (v3 post-mortem): each queue has only ~6
            # in-flight transfer semaphores -- an issue instruction that
            # re-arms a semaphore WAITS for its previous transfer to
            # complete, so the issue pipeline is completion-gated.  Use
            # FEW, BIG transfers (>=2KB per-partition lines) per queue,
            # in need-order, and keep in-loop writes off the queues that
            # must keep delivering bulk.

            # SYNC queue (first to start moving, ~8.6us): o=0/o=1 weights
            # then all of x in 2-block chunks, in need-order.  w16 block
            # ranges per o: o0=[0:1], o1=[1:3], o2=[3:4], o3=[4:6],
            # o4=[6:9], o5=[9:13], o6=[13:18], o7=[18:24], o8+=[24+6(o-8)].
            ld_w16(nc.sync, 0, 3)
            nc.sync.dma_start(x16_t[:, 0:512], x16_d[:, 0:512])
            nc.sync.dma_start(x16_t[:, 512:1024], x16_d[:, 512:1024])
            ld_x(nc.sync, 1, 3)
            for j in range(3, NUM_TIME_STEPS, 2):  # pairs (3,4),(5,6)..
                ld_x(nc.sync, j, min(j + 2, NUM_TIME_STEPS))

            # SCALAR queue (~10.4us): bias + early-need small weights.
            nc.scalar.dma_start(bias_t[:], bias_d[:])
            ld_w8(nc.scalar, 0, 2)
            ld_w16(nc.scalar, 3, 9)
            ld_w8(nc.scalar, 2, 8)
            ld_w16(nc.scalar, 9, 24)

            # GPSIMD queue: all late bulk, gated on x block 1 so it stays
            # out of the contended early window (earliest need is o=8 at
            # ~22us).  The tile scheduler is dependency-driven, so a
            # standalone gate op gets hoisted past the dma_starts; instead
            # poke one element of each gated destination region (reading
            # the x16 gate column) to create a real WAW dependency the
            # scheduler must honor.  The DMA then overwrites the poked
            # element with the true data.
            gx = x16_t[:, BC:BC + 1]
            nc.gpsimd.tensor_scalar_add(w16_t[:, 24, 0:1], gx, 0.0)
            ld_w16(nc.gpsimd, 24, 44)
            nc.gpsimd.tensor_scalar_add(w8_t[:, 8, 0, 0:1], gx, 0.0)
            ld_w8(nc.gpsimd, 8, 16)
            nc.gpsimd.tensor_scalar_add(w16_t[:, 44, 0:1], gx, 0.0)
            ld_w16(nc.gpsimd, 44, 64)
            nc.gpsimd.tensor_scalar_add(w16_t[:, 64, 0:1], gx, 0.0)
            ld_w16(nc.gpsimd, 64, 112)
            nc.gpsimd.tensor_scalar_add(w8_t[:, 16, 0, 0:1], gx, 0.0)
            ld_w8(nc.gpsimd, 16, 30)
            nc.gpsimd.tensor_scalar_add(w16_t[:, 112, 0:1], gx, 0.0)
            ld_w16(nc.gpsimd, 112, 168)

            # x8 derived on-device: e4m3(8 * x16) per block on DVE
            # (gpsimd measures ~13us per block for this op -- unusable;
            # per-pair tiles split DVE/ACT measured slower too).
            x8_t = xp8.tile([128, NUM_TIME_STEPS, BC], f8, tag="x8")

            def convert(j):
                nc.vector.tensor_scalar_mul(
                    x8_t[:, j, :], x16_t[:, j * BC:(j + 1) * BC], 8.0)

            convert(0)
            convert(1)

            inv = 1.0 / SCALE
            max_x8 = NUM_TIME_STEPS - TRI_BLOCK + 1  # highest x8 block read
            for o in range(NUM_TIME_STEPS):
                lo = max(0, o - TRI_BLOCK + 1)
                f16bl = _f16_blocks(o)
                k0 = int(_K16[o])
                ps = [psp.tile([128, 512], f32, tag="ps", name=f"ps{o}_{h}")
                      for h in range(2)]
                started = [False, False]
                n_units = (1 if o >= 2 else 0) + len(f16bl)
                unit = 0
                if o >= 2:
                    unit += 1
                    for h in range(2):
                        nc.tensor.matmul(
                            ps[h][:],
                            w8_t[:, o - 2, :, :],
                            x8_t[:, lo:lo + 2, h * 512:(h + 1) * 512],
                            start=True, stop=(unit == n_units),
                            perf_mode=mybir.MatmulPerfMode.DoubleRow)
                    started = [True, True]
                # x8[j] is first read at o = j+6 (as lo+1).  The x8 tile
                # is watermark-tracked: each DR matmul waits on the
                # latest x8 write issued before it in program order.
                # Issuing conv(o-5) HERE -- after this iteration's DR,
                # before the fp16 matmuls -- means DR(o) waits only
                # conv(o-6) from the previous iteration (which ran after
                # out-h1(o-2), a full iteration of slack), and conv(o-5)
                # itself runs on DVE while o's fp16 matmuls stream.
                if 2 <= o - 5 <= max_x8:
                    convert(o - 5)
                for idx, j in enumerate(f16bl):
                    unit += 1
                    for h in range(2):
                        nc.tensor.matmul(
                            ps[h][:],
                            w16_t[:, k0 + idx, :],
                            x16_t[:, j * BC + h * 512:j * BC + (h + 1) * 512],
                            start=not started[h], stop=(unit == n_units))
                    started = [True, True]
                # output stage split across engines: h0 on ACT, h1 on DVE
                out0 = op.tile([128, 512], f16, tag="oh0")
                out1 = op.tile([128, 512], f16, tag="oh1")
                nc.scalar.activation(out0[:], ps[0][:],
                                     mybir.ActivationFunctionType.Identity,
                                     bias=bias_t[:, o:o + 1], scale=inv)
                nc.vector.tensor_scalar(
                    out1[:], ps[1][:], inv, bias_t[:, o:o + 1],
                    mybir.AluOpType.mult, mybir.AluOpType.add)
                nc.scalar.dma_start(yT_d[o, :, 0:512], out0[:])
                nc.sync.dma_start(yT_d[o, :, 512:1024], out1[:])

    nc.compile()
    return nc


def _get_program():
    global _PROGRAM
    if _PROGRAM is None:
        _PROGRAM = _build_program()
    return _PROGRAM


def _pack_inputs(x, weight, bias, mask):
    x = np.asarray(x, dtype=np.float32)
    weight = np.asarray(weight, dtype=np.float32)
    bias = np.asarray(bias, dtype=np.float32)
    mask = np.asarray(mask)
    wm = weight * mask

    w16_flat = np.empty((128, N_F16 * 128), dtype=np.float16)
    k = 0
    for o in range(NUM_TIME_STEPS):
        for j in _f16_blocks(o):
            blk = wm[o * 128:(o + 1) * 128, j * 128:(j + 1) * 128]
            w16_flat[:, k * 128:(k + 1) * 128] = (blk.T * SCALE).astype(
                np.float16)
            k += 1

    w8_flat = np.empty((128, N_PAIR * 256), dtype=E4)
    for o in range(2, NUM_TIME_STEPS):
        lo = max(0, o - TRI_BLOCK + 1)
        p = o - 2
        for s, j in enumerate((lo, lo + 1)):
            blk = wm[o * 128:(o + 1) * 128, j * 128:(j + 1) * 128]
            w8_flat[:, p * 256 + s * 128:p * 256 + (s + 1) * 128] = (
                blk.T * SW).astype(E4)

    bias_t = np.ascontiguousarray(bias.reshape(NUM_TIME_STEPS, 128).T)

    x16 = x.astype(np.float16)
    in_maps = []
    for c in range(N_CORES):
        sl = slice(c * BC, (c + 1) * BC)
        x16c = np.ascontiguousarray(
            x16[sl].reshape(BC, NUM_TIME_STEPS, 128).transpose(2, 1, 0)
        ).reshape(128, NUM_TIME_STEPS * BC)
        in_maps.append({
            "x16": x16c,
            "w16": w16_flat.reshape(128, N_F16, 128),
            "w8": w8_flat.reshape(128, N_PAIR, 2, 128),
            "bias_t": bias_t,
        })
    return in_maps


def _run(inputs, trace=False):
    from concourse.bass_utils import run_bass_kernel_spmd

    nc = _get_program()
    in_maps = _pack_inputs(**inputs)
    res = run_bass_kernel_spmd(nc, in_maps, list(range(N_CORES)), trace=trace)

    y = np.empty((BATCH, OUT_SIZE), dtype=np.float32)
    for c in range(N_CORES):
        yTc = res.results[c]["yT"].astype(np.float32).reshape(OUT_SIZE, BC)
        y[c * BC:(c + 1) * BC] = yTc.T
    return y, res


def kernel(x, weight, bias, mask):
    y, _ = _run({"x": x, "weight": weight, "bias": bias, "mask": mask})
    return y


# revision 72
# speedup vs baseline: 1.1248x; 1.0080x over previous
"""CausalMaskedLinear Trainium2 kernel (v3: head/tail-optimized).

y = x @ (W * mask).T + b with a block-banded causal mask: output block o
(128 rows) attends to input blocks j in [o-7, o], so only 228 of the
1024 128x128 weight blocks are live.

Strategy: data-parallel over batch (8192/8 = 1024 rows per core),
weights/bias replicated.  Per output block o the two OLDEST band blocks
(j = lo, lo+1, for o >= 2) are computed in fp8 e4m3 via one DoubleRow
matmul (two 128-deep contractions per instruction, 2x PE rate); the
remaining blocks run in fp16.  Numerics (validated against the harness
seed): max/scale err 1.39e-2 < 2e-2 gate.

Scaling: e4m3's normal range starts at 2^-6, so x is quantized as
e4m3(8*x) and w as e4m3(256*w); fp16 blocks carry w*2048 so every
matmul contributes 2048*x*w to the shared PSUM accumulation.  Output
stage is split per 512-col half: h0 on the ACT engine
(Identity(psum/2048 + bias)), h1 on DVE (tensor_scalar mult+add), so
neither engine gates PSUM recycling.

v4 changes (trace-driven; v2 baseline 120.7us, v3 regressed to 126.6us):
- v2's head had a 12us PE-idle gap: x block 0 rode the latest-starting
  DMA queue behind bias while bulk x/w saturated the engines, and the
  idle re-throttled HAM to 1.2GHz.  v3 fixed the ordering but used many
  small transfers; each queue has only ~6 in-flight transfer
  semaphores, and an issue that re-arms one WAITS for the prior
  transfer, so the issue pipeline serialized and starved the mid-game.
- v4: few BIG transfers per queue in need-order.  sync queue (earliest
  to start) carries o0/o1 weights then all of x in 2-block chunks;
  scalar carries bias + o2..o7 weights; gpsimd carries all late bulk
  behind a data gate on x block 2.
- Warmup: 6x N=512 + 4x N=128 dummy matmuls end right when x block 0
  lands (~10.4us), keeping HAM warm without delaying real work.
- Output stage split per 512-col half across engines (h0 ACT, h1 DVE)
  and across queues (h0 scalar, h1 gpsimd) so the final o's output
  drains in ~2us instead of 6us queued behind 8MB of writes.
"""

import numpy as np
import ml_dtypes

NUM_TIME_STEPS = 32
IN_FEAT = 128
OUT_FEAT = 128
TRI_BLOCK = 8
BATCH = 8192
N_CORES = 8
BC = BATCH // N_CORES  # batch rows per core

IN_SIZE = NUM_TIME_STEPS * IN_FEAT
OUT_SIZE = NUM_TIME_STEPS * OUT_FEAT

SX = 8.0     # fp8 x scale
SW = 256.0   # fp8 w scale
SCALE = SX * SW  # 2048; fp16 w blocks carry w*SCALE

E4 = ml_dtypes.float8_e4m3  # matches mybir.dt.float8e4


def _band(o):
    return range(max(0, o - TRI_BLOCK + 1), o + 1)


# per-o split: o>=2 -> fp8 pair (lo, lo+1) + fp16 rest; o<2 -> all fp16
def _f16_blocks(o):
    bl = list(_band(o))
    return bl[2:] if o >= 2 else bl


N_F16 = sum(len(_f16_blocks(o)) for o in range(NUM_TIME_STEPS))  # 168
_K16 = np.cumsum([0] + [len(_f16_blocks(o)) for o in range(NUM_TIME_STEPS)])
N_PAIR = NUM_TIME_STEPS - 2  # 30

_PROGRAM = None


def _build_program():
    import concourse.bacc as bacc
    import concourse.bass as bass
    import concourse.mybir as mybir
    import concourse.tile as tile

    f32 = mybir.dt.float32
    f16 = mybir.dt.float16
    f8 = mybir.dt.float8e4

    nc = bacc.Bacc("TRN2", target_bir_lowering=False, debug=False,
                   enable_asserts=False)

    x16_d = nc.dram_tensor("x16", [128, NUM_TIME_STEPS * BC], f16,
                           kind="ExternalInput")
    w16_d = nc.dram_tensor("w16", [128, N_F16, 128], f16,
                           kind="ExternalInput")
    w8_d = nc.dram_tensor("w8", [128, N_PAIR, 2, 128], f8,
                          kind="ExternalInput")
    bias_d = nc.dram_tensor("bias_t", [128, NUM_TIME_STEPS], f32,
                            kind="ExternalInput")
    yT_d = nc.dram_tensor("yT", [NUM_TIME_STEPS, 128, BC], f16,
                          kind="ExternalOutput")

    with tile.TileContext(nc) as tc:
        with (
            tc.tile_pool(name="xp16", bufs=1) as xp16,
            tc.tile_pool(name="xp8", bufs=1) as xp8,
            tc.tile_pool(name="wp16", bufs=1) as wp16,
            tc.tile_pool(name="wp8", bufs=1) as wp8,
            tc.tile_pool(name="op", bufs=16) as op,
            tc.tile_pool(name="wmp", bufs=1) as wmp,
            tc.tile_pool(name="bp", bufs=1) as bp,
            tc.tile_pool(name="psp", bufs=8, space=bass.MemorySpace.PSUM) as psp,
        ):
            bias_t = bp.tile([128, NUM_TIME_STEPS], f32)

            # PE pre-warm: HAM un-throttles (1.2 -> 2.4 GHz) only after
            # ~3.4us sustained activity; sized to end right as x block 0
            # lands (~10.4us) so the PE never idles long enough to
            # re-throttle.  Also preload the ACT table set (~2.7us
            # one-time) before the first output op needs it.
            warm_in = wmp.tile([128, 512], f16, tag="warm")
            nc.gpsimd.memset(warm_in[:], 0.0)
            warm_out = wmp.tile([128, 1], f32, tag="warmo")
            nc.scalar.activation(warm_out[:], warm_in[:, :1],
                                 mybir.ActivationFunctionType.Identity,
                                 bias=0.0, scale=1.0)
            warm_ps = psp.tile([128, 512], f32, tag="ps")
            for _ in range(8):
                nc.tensor.matmul(warm_ps[:], warm_in[:, :128], warm_in[:],
                                 start=True, stop=True)
            for _ in range(4):
                nc.tensor.matmul(warm_ps[:, :128], warm_in[:, :128],
                                 warm_in[:, :128], start=True, stop=True)

            # big region-tracked tiles; per-block DMAs keep deps fine-grained
            x16_t = xp16.tile([128, NUM_TIME_STEPS * BC], f16, tag="x16")
            w16_t = wp16.tile([128, N_F16, 128], f16, tag="w16")
            w8_t = wp8.tile([128, N_PAIR, 2, 128], f8, tag="w8")

            def ld_x(eng, j0, j1):
                eng.dma_start(x16_t[:, j0 * BC:j1 * BC],
                              x16_d[:, j0 * BC:j1 * BC])

            def ld_w16(eng, k0, k1):
                eng.dma_start(w16_t[:, k0:k1, :], w16_d[:, k0:k1, :])

            def ld_w8(eng, p0, p1):
                eng.dma_start(w8_t[:, p0:p1, :, :], w8_d[:, p0:p1, :, :])

            # DMA design rules (v3 post-mortem): each queue has only ~6
            # in-flight transfer semaphores -- an issue instruction that
            # re-arms a semaphore WAITS for its previous transfer to
            # complete, so the issue pipeline is completion-gated.  Use
            # FEW, BIG transfers (>=2KB per-partition lines) per queue,
            # in need-order, and keep in-loop writes off the queues that
            # must keep delivering bulk.

            # SYNC queue (first to start moving, ~8.6us): o=0/o=1 weights
            # then all of x in 2-block chunks, in need-order.  w16 block
            # ranges per o: o0=[0:1], o1=[1:3], o2=[3:4], o3=[4:6],
            # o4=[6:9], o5=[9:13], o6=[13:18], o7=[18:24], o8+=[24+6(o-8)].
            # x block 0 first: the first-matmul critical path is x-bound
            # (w16[0:3] lands ~9.7us, x0 was landing ~12.5us behind it)
            nc.sync.dma_start(x16_t[:, 0:512], x16_d[:, 0:512])
            nc.sync.dma_start(x16_t[:, 512:1024], x16_d[:, 512:1024])
            ld_w16(nc.sync, 0, 3)
            ld_x(nc.sync, 1, 3)
            for j in range(3, NUM_TIME_STEPS, 2):  # pairs (3,4),(5,6)..
                ld_x(nc.sync, j, min(j + 2, NUM_TIME_STEPS))

            # SCALAR queue (~10.4us): bias + early-need small weights.
            nc.scalar.dma_start(bias_t[:], bias_d[:])
            ld_w8(nc.scalar, 0, 2)
            ld_w16(nc.scalar, 3, 9)
            ld_w8(nc.scalar, 2, 8)
            ld_w16(nc.scalar, 9, 24)

            # GPSIMD queue: all late bulk, gated on x block 1 so it stays
            # out of the contended early window (earliest need is o=8 at
            # ~22us).  The tile scheduler is dependency-driven, so a
            # standalone gate op gets hoisted past the dma_starts; instead
            # poke one element of each gated destination region (reading
            # the x16 gate column) to create a real WAW dependency the
            # scheduler must honor.  The DMA then overwrites the poked
            # element with the true data.
            gx = x16_t[:, BC:BC + 1]
            nc.gpsimd.tensor_scalar_add(w16_t[:, 24, 0:1], gx, 0.0)
            ld_w16(nc.gpsimd, 24, 44)
            nc.gpsimd.tensor_scalar_add(w8_t[:, 8, 0, 0:1], gx, 0.0)
            ld_w8(nc.gpsimd, 8, 16)
            nc.gpsimd.tensor_scalar_add(w16_t[:, 44, 0:1], gx, 0.0)
            ld_w16(nc.gpsimd, 44, 64)
            nc.gpsimd.tensor_scalar_add(w16_t[:, 64, 0:1], gx, 0.0)
            ld_w16(nc.gpsimd, 64, 112)
            nc.gpsimd.tensor_scalar_add(w8_t[:, 16, 0, 0:1], gx, 0.0)
            ld_w8(nc.gpsimd, 16, 30)
            nc.gpsimd.tensor_scalar_add(w16_t[:, 112, 0:1], gx, 0.0)
            ld_w16(nc.gpsimd, 112, 168)

            # x8 derived on-device: e4m3(8 * x16) per block on DVE
            # (gpsimd measures ~13us per block for this op -- unusable;
            # per-pair tiles split DVE/ACT measured slower too).
            x8_t = xp8.tile([128, NUM_TIME_STEPS, BC], f8, tag="x8")

            def convert(j):
                nc.vector.tensor_scalar_mul(
                    x8_t[:, j, :], x16_t[:, j * BC:(j + 1) * BC], 8.0)

            convert(0)
            convert(1)

            inv = 1.0 / SCALE
            max_x8 = NUM_TIME_STEPS - TRI_BLOCK + 1  # highest x8 block read
            for o in range(NUM_TIME_STEPS):
                # x8[j] is first read at o = j+6 (as lo+1).  The x8 tile
                # is watermark-tracked: each DR matmul waits on the
                # latest x8 write issued before it in program order, so
                # issue conv(j) at the top of iteration j+5 -- a full
                # iteration before its first reader, and none at all
                # during the o<7 ramp.
                if 2 <= o - 5 <= max_x8:
                    convert(o - 5)
                lo = max(0, o - TRI_BLOCK + 1)
                f16bl = _f16_blocks(o)
                k0 = int(_K16[o])
                ps = [psp.tile([128, 512], f32, tag="ps", name=f"ps{o}_{h}")
                      for h in range(2)]
                started = [False, False]
                n_units = (1 if o >= 2 else 0) + len(f16bl)
                unit = 0
                if o >= 2:
                    unit += 1
                    for h in range(2):
                        nc.tensor.matmul(
                            ps[h][:],
                            w8_t[:, o - 2, :, :],
                            x8_t[:, lo:lo + 2, h * 512:(h + 1) * 512],
                            start=True, stop=(unit == n_units),
                            perf_mode=mybir.MatmulPerfMode.DoubleRow)
                    started = [True, True]
                for idx, j in enumerate(f16bl):
                    unit += 1
                    for h in range(2):
                        nc.tensor.matmul(
                            ps[h][:],
                            w16_t[:, k0 + idx, :],
                            x16_t[:, j * BC + h * 512:j * BC + (h + 1) * 512],
                            start=not started[h], stop=(unit == n_units))
                    started = [True, True]
                # output stage split across engines: h0 on ACT, h1 on DVE
                out0 = op.tile([128, 512], f16, tag="oh0")
                out1 = op.tile([128, 512], f16, tag="oh1")
                nc.scalar.activation(out0[:], ps[0][:],
                                     mybir.ActivationFunctionType.Identity,
                                     bias=bias_t[:, o:o + 1], scale=inv)
                nc.vector.tensor_scalar(
                    out1[:], ps[1][:], inv, bias_t[:, o:o + 1],
                    mybir.AluOpType.mult, mybir.AluOpType.add)
                nc.scalar.dma_start(yT_d[o, :, 0:512], out0[:])
                nc.sync.dma_start(yT_d[o, :, 512:1024], out1[:])

    nc.compile()
    return nc


def _get_program():
    global _PROGRAM
    if _PROGRAM is None:
        _PROGRAM = _build_program()
    return _PROGRAM


def _pack_inputs(x, weight, bias, mask):
    x = np.asarray(x, dtype=np.float32)
    weight = np.asarray(weight, dtype=np.float32)
    bias = np.asarray(bias, dtype=np.float32)
    mask = np.asarray(mask)
    wm = weight * mask

    w16_flat = np.empty((128, N_F16 * 128), dtype=np.float16)
    k = 0
    for o in range(NUM_TIME_STEPS):
        for j in _f16_blocks(o):
            blk = wm[o * 128:(o + 1) * 128, j * 128:(j + 1) * 128]
            w16_flat[:, k * 128:(k + 1) * 128] = (blk.T * SCALE).astype(
                np.float16)
            k += 1

    w8_flat = np.empty((128, N_PAIR * 256), dtype=E4)
    for o in range(2, NUM_TIME_STEPS):
        lo = max(0, o - TRI_BLOCK + 1)
        p = o - 2
        for s, j in enumerate((lo, lo + 1)):
            blk = wm[o * 128:(o + 1) * 128, j * 128:(j + 1) * 128]
            w8_flat[:, p * 256 + s * 128:p * 256 + (s + 1) * 128] = (
                blk.T * SW).astype(E4)

    bias_t = np.ascontiguousarray(bias.reshape(NUM_TIME_STEPS, 128).T)

    x16 = x.astype(np.float16)
    in_maps = []
    for c in range(N_CORES):
        sl = slice(c * BC, (c + 1) * BC)
        x16c = np.ascontiguousarray(
            x16[sl].reshape(BC, NUM_TIME_STEPS, 128).transpose(2, 1, 0)
        ).reshape(128, NUM_TIME_STEPS * BC)
        in_maps.append({
            "x16": x16c,
            "w16": w16_flat.reshape(128, N_F16, 128),
            "w8": w8_flat.reshape(128, N_PAIR, 2, 128),
            "bias_t": bias_t,
        })
    return in_maps


def _run(inputs, trace=False):
    from concourse.bass_utils import run_bass_kernel_spmd

    nc = _get_program()
    in_maps = _pack_inputs(**inputs)
    res = run_bass_kernel_spmd(nc, in_maps, list(range(N_CORES)), trace=trace)

    y = np.empty((BATCH, OUT_SIZE), dtype=np.float32)
    for c in range(N_CORES):
        yTc = res.results[c]["yT"].astype(np.float32).reshape(OUT_SIZE, BC)
        y[c * BC:(c + 1) * BC] = yTc.T
    return y, res


def kernel(x, weight, bias, mask):
    y, _ = _run({"x": x, "weight": weight, "bias": bias, "mask": mask})
    return y


# revision 73
# speedup vs baseline: 1.1300x; 1.0047x over previous
"""CausalMaskedLinear Trainium2 kernel (v3: head/tail-optimized).

y = x @ (W * mask).T + b with a block-banded causal mask: output block o
(128 rows) attends to input blocks j in [o-7, o], so only 228 of the
1024 128x128 weight blocks are live.

Strategy: data-parallel over batch (8192/8 = 1024 rows per core),
weights/bias replicated.  Per output block o the two OLDEST band blocks
(j = lo, lo+1, for o >= 2) are computed in fp8 e4m3 via one DoubleRow
matmul (two 128-deep contractions per instruction, 2x PE rate); the
remaining blocks run in fp16.  Numerics (validated against the harness
seed): max/scale err 1.39e-2 < 2e-2 gate.

Scaling: e4m3's normal range starts at 2^-6, so x is quantized as
e4m3(8*x) and w as e4m3(256*w); fp16 blocks carry w*2048 so every
matmul contributes 2048*x*w to the shared PSUM accumulation.  Output
stage is split per 512-col half: h0 on the ACT engine
(Identity(psum/2048 + bias)), h1 on DVE (tensor_scalar mult+add), so
neither engine gates PSUM recycling.

v4 changes (trace-driven; v2 baseline 120.7us, v3 regressed to 126.6us):
- v2's head had a 12us PE-idle gap: x block 0 rode the latest-starting
  DMA queue behind bias while bulk x/w saturated the engines, and the
  idle re-throttled HAM to 1.2GHz.  v3 fixed the ordering but used many
  small transfers; each queue has only ~6 in-flight transfer
  semaphores, and an issue that re-arms one WAITS for the prior
  transfer, so the issue pipeline serialized and starved the mid-game.
- v4: few BIG transfers per queue in need-order.  sync queue (earliest
  to start) carries o0/o1 weights then all of x in 2-block chunks;
  scalar carries bias + o2..o7 weights; gpsimd carries all late bulk
  behind a data gate on x block 2.
- Warmup: 6x N=512 + 4x N=128 dummy matmuls end right when x block 0
  lands (~10.4us), keeping HAM warm without delaying real work.
- Output stage split per 512-col half across engines (h0 ACT, h1 DVE)
  and across queues (h0 scalar, h1 gpsimd) so the final o's output
  drains in ~2us instead of 6us queued behind 8MB of writes.
"""

import numpy as np
import ml_dtypes

NUM_TIME_STEPS = 32
IN_FEAT = 128
OUT_FEAT = 128
TRI_BLOCK = 8
BATCH = 8192
N_CORES = 8
BC = BATCH // N_CORES  # batch rows per core

IN_SIZE = NUM_TIME_STEPS * IN_FEAT
OUT_SIZE = NUM_TIME_STEPS * OUT_FEAT

SX = 8.0     # fp8 x scale
SW = 256.0   # fp8 w scale
SCALE = SX * SW  # 2048; fp16 w blocks carry w*SCALE

E4 = ml_dtypes.float8_e4m3  # matches mybir.dt.float8e4


def _band(o):
    return range(max(0, o - TRI_BLOCK + 1), o + 1)


# per-o split: o>=2 -> fp8 pair (lo, lo+1) + fp16 rest; o<2 -> all fp16
def _f16_blocks(o):
    bl = list(_band(o))
    return bl[2:] if o >= 2 else bl


N_F16 = sum(len(_f16_blocks(o)) for o in range(NUM_TIME_STEPS))  # 168
_K16 = np.cumsum([0] + [len(_f16_blocks(o)) for o in range(NUM_TIME_STEPS)])
N_PAIR = NUM_TIME_STEPS - 2  # 30

_PROGRAM = None


def _build_program():
    import concourse.bacc as bacc
    import concourse.bass as bass
    import concourse.mybir as mybir
    import concourse.tile as tile

    f32 = mybir.dt.float32
    f16 = mybir.dt.float16
    f8 = mybir.dt.float8e4

    nc = bacc.Bacc("TRN2", target_bir_lowering=False, debug=False,
                   enable_asserts=False)

    x16_d = nc.dram_tensor("x16", [128, NUM_TIME_STEPS * BC], f16,
                           kind="ExternalInput")
    w16_d = nc.dram_tensor("w16", [128, N_F16, 128], f16,
                           kind="ExternalInput")
    w8_d = nc.dram_tensor("w8", [128, N_PAIR, 2, 128], f8,
                          kind="ExternalInput")
    bias_d = nc.dram_tensor("bias_t", [128, NUM_TIME_STEPS], f32,
                            kind="ExternalInput")
    yT_d = nc.dram_tensor("yT", [NUM_TIME_STEPS, 128, BC], f16,
                          kind="ExternalOutput")

    with tile.TileContext(nc) as tc:
        with (
            tc.tile_pool(name="xp16", bufs=1) as xp16,
            tc.tile_pool(name="xp8", bufs=1) as xp8,
            tc.tile_pool(name="wp16", bufs=1) as wp16,
            tc.tile_pool(name="wp8", bufs=1) as wp8,
            tc.tile_pool(name="op", bufs=16) as op,
            tc.tile_pool(name="wmp", bufs=1) as wmp,
            tc.tile_pool(name="bp", bufs=1) as bp,
            tc.tile_pool(name="psp", bufs=8, space=bass.MemorySpace.PSUM) as psp,
        ):
            bias_t = bp.tile([128, NUM_TIME_STEPS], f32)

            # PE pre-warm: HAM un-throttles (1.2 -> 2.4 GHz) only after
            # ~3.4us sustained activity; sized to end right as x block 0
            # lands (~10.4us) so the PE never idles long enough to
            # re-throttle.  Also preload the ACT table set (~2.7us
            # one-time) before the first output op needs it.
            warm_in = wmp.tile([128, 512], f16, tag="warm")
            nc.gpsimd.memset(warm_in[:], 0.0)
            warm_out = wmp.tile([128, 1], f32, tag="warmo")
            nc.scalar.activation(warm_out[:], warm_in[:, :1],
                                 mybir.ActivationFunctionType.Identity,
                                 bias=0.0, scale=1.0)
            warm_ps = psp.tile([128, 512], f32, tag="ps")
            for _ in range(8):
                nc.tensor.matmul(warm_ps[:], warm_in[:, :128], warm_in[:],
                                 start=True, stop=True)
            for _ in range(4):
                nc.tensor.matmul(warm_ps[:, :128], warm_in[:, :128],
                                 warm_in[:, :128], start=True, stop=True)

            # big region-tracked tiles; per-block DMAs keep deps fine-grained
            x16_t = xp16.tile([128, NUM_TIME_STEPS * BC], f16, tag="x16")
            w16_t = wp16.tile([128, N_F16, 128], f16, tag="w16")
            w8_t = wp8.tile([128, N_PAIR, 2, 128], f8, tag="w8")

            def ld_x(eng, j0, j1):
                eng.dma_start(x16_t[:, j0 * BC:j1 * BC],
                              x16_d[:, j0 * BC:j1 * BC])

            def ld_w16(eng, k0, k1):
                eng.dma_start(w16_t[:, k0:k1, :], w16_d[:, k0:k1, :])

            def ld_w8(eng, p0, p1):
                eng.dma_start(w8_t[:, p0:p1, :, :], w8_d[:, p0:p1, :, :])

            # DMA design rules (v3 post-mortem): each queue has only ~6
            # in-flight transfer semaphores -- an issue instruction that
            # re-arms a semaphore WAITS for its previous transfer to
            # complete, so the issue pipeline is completion-gated.  Use
            # FEW, BIG transfers (>=2KB per-partition lines) per queue,
            # in need-order, and keep in-loop writes off the queues that
            # must keep delivering bulk.

            # SYNC queue (first to start moving, ~8.6us): o=0/o=1 weights
            # then all of x in 2-block chunks, in need-order.  w16 block
            # ranges per o: o0=[0:1], o1=[1:3], o2=[3:4], o3=[4:6],
            # o4=[6:9], o5=[9:13], o6=[13:18], o7=[18:24], o8+=[24+6(o-8)].
            ld_w16(nc.sync, 0, 3)
            nc.sync.dma_start(x16_t[:, 0:512], x16_d[:, 0:512])
            nc.sync.dma_start(x16_t[:, 512:1024], x16_d[:, 512:1024])
            ld_x(nc.sync, 1, 3)
            for j in range(3, NUM_TIME_STEPS, 2):  # pairs (3,4),(5,6)..
                ld_x(nc.sync, j, min(j + 2, NUM_TIME_STEPS))

            # SCALAR queue (~10.4us): bias + early-need small weights.
            nc.scalar.dma_start(bias_t[:], bias_d[:])
            ld_w8(nc.scalar, 0, 2)
            ld_w16(nc.scalar, 3, 9)
            ld_w8(nc.scalar, 2, 8)
            ld_w16(nc.scalar, 9, 24)

            # GPSIMD queue: all late bulk, gated on x block 1 so it stays
            # out of the contended early window (earliest need is o=8 at
            # ~22us).  The tile scheduler is dependency-driven, so a
            # standalone gate op gets hoisted past the dma_starts; instead
            # poke one element of each gated destination region (reading
            # the x16 gate column) to create a real WAW dependency the
            # scheduler must honor.  The DMA then overwrites the poked
            # element with the true data.
            gx = x16_t[:, BC:BC + 1]
            nc.gpsimd.tensor_scalar_add(w16_t[:, 24, 0:1], gx, 0.0)
            ld_w16(nc.gpsimd, 24, 44)
            nc.gpsimd.tensor_scalar_add(w8_t[:, 8, 0, 0:1], gx, 0.0)
            ld_w8(nc.gpsimd, 8, 16)
            nc.gpsimd.tensor_scalar_add(w16_t[:, 44, 0:1], gx, 0.0)
            ld_w16(nc.gpsimd, 44, 64)
            nc.gpsimd.tensor_scalar_add(w16_t[:, 64, 0:1], gx, 0.0)
            ld_w16(nc.gpsimd, 64, 112)
            nc.gpsimd.tensor_scalar_add(w8_t[:, 16, 0, 0:1], gx, 0.0)
            ld_w8(nc.gpsimd, 16, 30)
            nc.gpsimd.tensor_scalar_add(w16_t[:, 112, 0:1], gx, 0.0)
            ld_w16(nc.gpsimd, 112, 168)

            # x8 derived on-device: e4m3(8 * x16) per block on DVE
            # (gpsimd measures ~13us per block for this op -- unusable;
            # per-pair tiles split DVE/ACT measured slower too).
            x8_t = xp8.tile([128, NUM_TIME_STEPS, BC], f8, tag="x8")

            def convert(j):
                nc.vector.tensor_scalar_mul(
                    x8_t[:, j, :], x16_t[:, j * BC:(j + 1) * BC], 8.0)

            convert(0)
            convert(1)

            inv = 1.0 / SCALE
            max_x8 = NUM_TIME_STEPS - TRI_BLOCK + 1  # highest x8 block read
            for o in range(NUM_TIME_STEPS):
                # x8[j] is first read at o = j+6 (as lo+1).  The x8 tile
                # is watermark-tracked: each DR matmul waits on the
                # latest x8 write issued before it in program order, so
                # issue conv(j) at the top of iteration j+5 -- a full
                # iteration before its first reader, and none at all
                # during the o<7 ramp.
                if 2 <= o - 5 <= max_x8:
                    convert(o - 5)
                lo = max(0, o - TRI_BLOCK + 1)
                f16bl = _f16_blocks(o)
                k0 = int(_K16[o])
                ps = [psp.tile([128, 512], f32, tag="ps", name=f"ps{o}_{h}")
                      for h in range(2)]
                started = [False, False]
                n_units = (1 if o >= 2 else 0) + len(f16bl)
                unit = 0
                if o >= 2:
                    unit += 1
                    for h in range(2):
                        nc.tensor.matmul(
                            ps[h][:],
                            w8_t[:, o - 2, :, :],
                            x8_t[:, lo:lo + 2, h * 512:(h + 1) * 512],
                            start=True, stop=(unit == n_units),
                            perf_mode=mybir.MatmulPerfMode.DoubleRow)
                    started = [True, True]
                for idx, j in enumerate(f16bl):
                    unit += 1
                    for h in range(2):
                        nc.tensor.matmul(
                            ps[h][:],
                            w16_t[:, k0 + idx, :],
                            x16_t[:, j * BC + h * 512:j * BC + (h + 1) * 512],
                            start=not started[h], stop=(unit == n_units))
                    started = [True, True]
                # output stage split across engines: h0 on ACT, h1 on DVE
                out0 = op.tile([128, 512], f16, tag="oh0")
                out1 = op.tile([128, 512], f16, tag="oh1")
                nc.scalar.activation(out0[:], ps[0][:],
                                     mybir.ActivationFunctionType.Identity,
                                     bias=bias_t[:, o:o + 1], scale=inv)
                nc.vector.tensor_scalar(
                    out1[:], ps[1][:], inv, bias_t[:, o:o + 1],
                    mybir.AluOpType.mult, mybir.AluOpType.add)
                nc.scalar.dma_start(yT_d[o, :, 0:512], out0[:])
                nc.sync.dma_start(yT_d[o, :, 512:1024], out1[:])

    nc.compile()
    return nc


def _get_program():
    global _PROGRAM
    if _PROGRAM is None:
        _PROGRAM = _build_program()
    return _PROGRAM


def _pack_inputs(x, weight, bias, mask):
    x = np.asarray(x, dtype=np.float32)
    weight = np.asarray(weight, dtype=np.float32)
    bias = np.asarray(bias, dtype=np.float32)
    mask = np.asarray(mask)
    wm = weight * mask

    w16_flat = np.empty((128, N_F16 * 128), dtype=np.float16)
    k = 0
    for o in range(NUM_TIME_STEPS):
        for j in _f16_blocks(o):
            blk = wm[o * 128:(o + 1) * 128, j * 128:(j + 1) * 128]
            w16_flat[:, k * 128:(k + 1) * 128] = (blk.T * SCALE).astype(
                np.float16)
            k += 1

    w8_flat = np.empty((128, N_PAIR * 256), dtype=E4)
    for o in range(2, NUM_TIME_STEPS):
        lo = max(0, o - TRI_BLOCK + 1)
        p = o - 2
        for s, j in enumerate((lo, lo + 1)):
            blk = wm[o * 128:(o + 1) * 128, j * 128:(j + 1) * 128]
            w8_flat[:, p * 256 + s * 128:p * 256 + (s + 1) * 128] = (
                blk.T * SW).astype(E4)

    bias_t = np.ascontiguousarray(bias.reshape(NUM_TIME_STEPS, 128).T)

    x16 = x.astype(np.float16)
    in_maps = []
    for c in range(N_CORES):
        sl = slice(c * BC, (c + 1) * BC)
        x16c = np.ascontiguousarray(
            x16[sl].reshape(BC, NUM_TIME_STEPS, 128).transpose(2, 1, 0)
        ).reshape(128, NUM_TIME_STEPS * BC)
        in_maps.append({
            "x16": x16c,
            "w16": w16_flat.reshape(128, N_F16, 128),
            "w8": w8_flat.reshape(128, N_PAIR, 2, 128),
            "bias_t": bias_t,
        })
    return in_maps


def _run(inputs, trace=False):
    from concourse.bass_utils import run_bass_kernel_spmd

    nc = _get_program()
    in_maps = _pack_inputs(**inputs)
    res = run_bass_kernel_spmd(nc, in_maps, list(range(N_CORES)), trace=trace)

    y = np.empty((BATCH, OUT_SIZE), dtype=np.float32)
    for c in range(N_CORES):
        yTc = res.results[c]["yT"].astype(np.float32).reshape(OUT_SIZE, BC)
        y[c * BC:(c + 1) * BC] = yTc.T
    return y, res


def kernel(x, weight, bias, mask):
    y, _ = _run({"x": x, "weight": weight, "bias": bias, "mask": mask})
    return y
